# revision 12
# baseline (speedup 1.0000x reference)
"""Trainium2 Bass kernel for dual-attention (DisKT-style) nn module.

Math per (batch, head) with S=1024, dk=64, all on-chip in [k, q] layout:
    sT       = (k_h @ q_h^T)            (+ -1e30 on causal-dead diag block)
    E1T      = exp(sT / 8)              (causally-dead region never computed)
    r1[q]    = sum_k E1T[k, q]          (ones^T @ E1T, PSUM broadcast rows)
    p1       = E1T * rec1[q]            (bf16 everywhere -> DVE 2x mode)
    e2''     = exp(p1) - 1              (bf16 sub -> DVE 4x mode; the "+1" of
                                         every key is an exact rank-1 vtot fixup)
    outT     = (cm*v)^T @ e2''          (cm pre-masked on host, bf16 v)
    r2       = 1024 + cmrep^T @ e2''
    out      = (outT + vtot) * (1/r2) ;  out[:, q=0] = 0
Outputs are produced as [d, q] and transposed back on the host.

exp1 per chunk runs on a configurable engine: 'act' = ScalarE spline exp,
'gps'/'dve' = Schraudolph bit-trick exp (i16 = s*A + B, bitcast as bf16;
-1e30-masked scores saturate to 0x8000 = -0.0). This offloads the
Act engine, which otherwise becomes the bottleneck at ~9.7us/block.

Live (causal) regions are stored PACKED; counter-mask folded into the PV
weights host-side. Emission is software-pipelined with a 2-block skew
(A(n) | D(n-2) | C(n-1)); D-work is interleaved between QK groups so the
PE never stalls on slow exp1 consumers.

Sharding: data-parallel over batch, B=16 -> 2 per core on 8 cores.
"""

import math

import numpy as np
import ml_dtypes

import concourse.bass as bass
import concourse.mybir as mybir
import concourse.tile as tile
from concourse import bacc
from concourse.bass_utils import run_bass_kernel_spmd

B, S, D, H = 16, 1024, 512, 8
DK = D // H           # 64
NCORES = 8
BLOC = B // NCORES    # 2 batches per core
NCH = S // 128        # 8 k-chunks of 128
F32 = mybir.dt.float32
BF16 = mybir.dt.bfloat16
I16 = mybir.dt.int16
NPBF16 = ml_dtypes.bfloat16

LIVE = [S - 128 * c for c in range(NCH)]          # live width per chunk
PACK = sum(LIVE)                                  # 4608

# ---------------- tuning knobs (test.py may flip before build) -------------
# packing order of chunks in the e1/tmp/e2 tiles: chosen so each exp1 group
# is contiguous and bin-packs to <=1024 columns (one scores-psum tile)
PACK_ORDER = [0, 4, 1, 7, 2, 6, 3, 5]
OFF = {}
_o = 0
for _c in PACK_ORDER:
    OFF[_c] = _o
    _o += LIVE[_c]
# QK groups: chunks sharing one scores-psum tile + one exp1 call.
# Must be adjacent in PACK_ORDER; group width <= 1024.
QK_GROUPS = [[0], [4], [1, 7], [2, 6], [3, 5]]
# exp1 engine per group: 'act' = ScalarE spline exp, 'dve' = Schraudolph
# bit-trick on VectorE (GPSIMD cannot read PSUM)
EXP1_ENGINE = ["act", "act", "act", "act", "act"]
# p1-multiply engine per chunk ('dve' | 'gps'); GpSimd is slow (~2.7ns/col)
# but otherwise idle, DVE does 2x-rate bf16
MUL_ENGINE = {0: "gps", 1: "gps", 4: "gps", 5: "gps",
              2: "dve", 3: "dve", 6: "dve", 7: "dve"}
# Schraudolph constants for bf16-bit exp(x/8): i16 = s * SCH_A + SCH_B
SCH_A = (128.0 / math.log(2.0)) / 8.0     # 23.0831
SCH_B = 16256.0 - 7.0                     # (127<<7) minus sawtooth centering
# split of the phase-C (mul/exp2/sub) work into pieces for pipelining
C_PIECES = 2
TRACE = False
LAST_RESULTS = None


def build_nc(debug=False):
    nc = bacc.Bacc("TRN2", target_bir_lowering=False, debug=debug)
    AF = mybir.ActivationFunctionType
    ALU = mybir.AluOpType

    qt_d = nc.dram_tensor("qt", [BLOC, H, DK, S], BF16, kind="ExternalInput")
    kt_d = nc.dram_tensor("kt", [BLOC, H, DK, S], BF16, kind="ExternalInput")
    # (1-cm)*[v1|v2] per (b, h, chunk), bf16
    vcat_d = nc.dram_tensor(
        "vcat", [BLOC, H, NCH, 128, 128], BF16, kind="ExternalInput"
    )
    vtot_d = nc.dram_tensor("vtot", [BLOC, H, 128], F32, kind="ExternalInput")
    # (1-cm) replicated across columns, per (b, chunk): r2 matmul weights
    cmrep_d = nc.dram_tensor("cmrep", [BLOC, NCH, 128, 128], BF16, kind="ExternalInput")
    ind_d = nc.dram_tensor("ind", [NCH, S], BF16, kind="ExternalInput")
    cnt_d = nc.dram_tensor("cnt", [NCH, 128], BF16, kind="ExternalInput")
    dmask_d = nc.dram_tensor("dmask", [128, 128], BF16, kind="ExternalInput")
    ident_d = nc.dram_tensor("ident", [128, 128], BF16, kind="ExternalInput")
    ones_d = nc.dram_tensor("onesd", [128, 128], BF16, kind="ExternalInput")
    out1_d = nc.dram_tensor("out1t", [BLOC, D, S], F32, kind="ExternalOutput")
    out2_d = nc.dram_tensor("out2t", [BLOC, D, S], F32, kind="ExternalOutput")

    def bank_pieces(p0, p1):
        """split [p0, p1) at 512-aligned psum bank boundaries"""
        out = []
        p = p0
        while p < p1:
            end = min(p1, (p // 512 + 1) * 512)
            out.append((p, end))
            p = end
        return out

    # phase-C column split of [0, PACK)
    csplit = []
    step = (PACK + C_PIECES - 1) // C_PIECES
    x = 0
    while x < PACK:
        csplit.append((x, min(PACK, x + step)))
        x += step

    with tile.TileContext(nc) as tc:
        with (
            tc.tile_pool(name="consts", bufs=1) as consts,
            tc.tile_pool(name="qk", bufs=3) as qkp,
            tc.tile_pool(name="vc", bufs=3) as vcp,
            tc.tile_pool(name="e1", bufs=2) as e1p,
            tc.tile_pool(name="e2", bufs=3) as e2p,
            tc.tile_pool(name="tmp", bufs=2) as tmpp,
            tc.tile_pool(name="rc", bufs=2) as rcp,
            tc.tile_pool(name="outs", bufs=2) as outp,
            tc.tile_pool(name="sc_ps", bufs=1, space="PSUM") as sc_psp,
            tc.tile_pool(name="r_ps", bufs=1, space="PSUM") as r_psp,
            tc.tile_pool(name="o_ps", bufs=1, space="PSUM") as o_psp,
        ):
            vtot_sb = consts.tile([128, BLOC * H], F32)
            nc.sync.dma_start(out=vtot_sb, in_=vtot_d[:].rearrange("b h d -> d (b h)"))
            dm_sb = consts.tile([128, 128], BF16)
            nc.sync.dma_start(out=dm_sb, in_=dmask_d[:, :])
            id_sb = consts.tile([128, 128], BF16)
            nc.sync.dma_start(out=id_sb, in_=ident_d[:, :])
            ones_sb = consts.tile([128, 128], BF16)
            nc.sync.dma_start(out=ones_sb, in_=ones_d[:, :])
            ind_sb = consts.tile([NCH, S], BF16)
            nc.sync.dma_start(out=ind_sb, in_=ind_d[:, :])
            cnt_sb = consts.tile([NCH, 128], BF16)
            nc.sync.dma_start(out=cnt_sb, in_=cnt_d[:, :])
            cmrep_sb = consts.tile([128, BLOC, NCH, 128], BF16)
            nc.sync.dma_start(
                out=cmrep_sb, in_=cmrep_d[:].rearrange("b c p j -> p b c j")
            )

            NB = BLOC * H
            st = [dict() for _ in range(NB)]

            def eng_of(name):
                return {"act": nc.scalar, "gps": nc.gpsimd, "dve": nc.vector}[name]

            def emit_qk_group(blk, grp):
                """QK + diag-mask matmuls for a chunk group -> exp1 -> e1."""
                s = st[blk]
                qt_sb, kt_sb, e1 = s["qt"], s["kt"], s["e1"]
                gw = sum(LIVE[c] for c in grp)
                assert gw <= 1024
                sps = sc_psp.tile([128, S], F32, tag="sc")
                loc = 0
                for c in grp:
                    q0 = 128 * c
                    for n0 in range(0, LIVE[c], 512):
                        w = min(512, LIVE[c] - n0)
                        nc.tensor.matmul(
                            sps[:, loc + n0:loc + n0 + w],
                            lhsT=kt_sb[:, q0:q0 + 128],
                            rhs=qt_sb[:, q0 + n0:q0 + n0 + w],
                            start=True,
                            stop=False,
                            skip_group_check=True,
                        )
                    # causal: += I^T @ dmask adds -1e30 above diag
                    nc.tensor.matmul(
                        sps[:, loc:loc + 128],
                        lhsT=id_sb,
                        rhs=dm_sb,
                        start=False,
                        stop=True,
                        skip_group_check=True,
                    )
                    loc += LIVE[c]
                o0 = OFF[grp[0]]
                eng = EXP1_ENGINE[QK_GROUPS.index(grp)]
                if eng == "act":
                    nc.scalar.activation(
                        e1[:, o0:o0 + gw], sps[:, 0:gw], AF.Exp, scale=0.125
                    )
                else:
                    # Schraudolph: i16 = round(s*A + B); bits are bf16 exp(s/8)
                    e1i = e1[:, o0:o0 + gw].bitcast(I16)
                    eng_of(eng).tensor_scalar(
                        e1i, sps[:, 0:gw], SCH_A, SCH_B,
                        ALU.mult, ALU.add,
                    )

            def emit_r1(blk, chunks, start, stop):
                s = st[blk]
                e1, r1ps = s["e1"], s["r1ps"]
                for i, c in enumerate(chunks):
                    q0 = 128 * c
                    pieces = bank_pieces(q0, S)
                    for j, (p0, p1) in enumerate(pieces):
                        # start=True on every piece of chunk 0: it is the
                        # first writer of each psum column range
                        nc.tensor.matmul(
                            r1ps[:, p0:p1],
                            lhsT=ones_sb,
                            rhs=e1[:, OFF[c] + p0 - q0:OFF[c] + p1 - q0],
                            start=(c == 0),
                            stop=(stop and i == len(chunks) - 1
                                  and j == len(pieces) - 1),
                            skip_group_check=True,
                        )

            def emit_rec1(blk):
                s = st[blk]
                rec1 = rcp.tile([128, S], F32, tag="rec1")
                nc.vector.reciprocal_approx_fast(out=rec1, in_=s["r1ps"][:, 0:S])
                nc.vector.memset(rec1[:, 0:1], 0.0)
                s["rec1"] = rec1

            def chunks_in(x0, x1):
                """chunks whose packed range intersects [x0, x1)"""
                out = []
                for c in PACK_ORDER:
                    a, b_ = max(x0, OFF[c]), min(x1, OFF[c] + LIVE[c])
                    if a < b_:
                        out.append((c, a, b_))
                return out

            def emit_C(blk, piece):
                """p1 mul -> exp2 -> sub for one packed column piece."""
                s = st[blk]
                x0, x1 = csplit[piece]
                e1, rec1 = s["e1"], s["rec1"]
                if piece == 0:
                    s["tmp"] = tmpp.tile([128, PACK], F32, tag="tmp", name="tmp")
                    s["e2"] = e2p.tile([128, PACK], BF16, tag="e2", name="e2")
                tmp, e2 = s["tmp"], s["e2"]
                for c, a, b_ in chunks_in(x0, x1):
                    q0 = 128 * c + (a - OFF[c])
                    eng_of(MUL_ENGINE[c]).tensor_tensor(
                        tmp[:, a:b_], e1[:, a:b_], rec1[:, q0:q0 + (b_ - a)],
                        ALU.mult,
                    )
                nc.scalar.activation(tmp[:, x0:x1], tmp[:, x0:x1], AF.Exp)
                nc.vector.tensor_scalar_add(e2[:, x0:x1], tmp[:, x0:x1], -1.0)

            def emit_D_start(blk):
                s = st[blk]
                s["otps"] = o_psp.tile([128, S], F32, tag="ot", name="otps")
                r2ps = r_psp.tile([128, S], F32, tag="r2")
                s["r2ps"] = r2ps
                for p0, p1 in bank_pieces(0, S):
                    nc.tensor.matmul(
                        r2ps[:, p0:p1],
                        lhsT=cnt_sb,
                        rhs=ind_sb[:, p0:p1],
                        start=True,
                        stop=False,
                        skip_group_check=True,
                    )

            def emit_pv(blk, chunks, stop):
                bi, h = divmod(blk, H)
                s = st[blk]
                e2, vc_sb, otps = s["e2"], s["vc"], s["otps"]
                for i, c in enumerate(chunks):
                    q0 = 128 * c
                    pieces = bank_pieces(q0, S)
                    for j, (p0, p1) in enumerate(pieces):
                        nc.tensor.matmul(
                            otps[:, p0:p1],
                            lhsT=vc_sb[:, c, :],
                            rhs=e2[:, OFF[c] + p0 - q0:OFF[c] + p1 - q0],
                            start=(c == 0),
                            stop=(stop and i == len(chunks) - 1
                                  and j == len(pieces) - 1),
                            skip_group_check=True,
                        )

            def emit_r2(blk, chunks, stop):
                bi, h = divmod(blk, H)
                s = st[blk]
                e2, r2ps = s["e2"], s["r2ps"]
                for i, c in enumerate(chunks):
                    q0 = 128 * c
                    pieces = bank_pieces(q0, S)
                    for j, (p0, p1) in enumerate(pieces):
                        nc.tensor.matmul(
                            r2ps[:, p0:p1],
                            lhsT=cmrep_sb[:, bi, c, :],
                            rhs=e2[:, OFF[c] + p0 - q0:OFF[c] + p1 - q0],
                            start=False,
                            stop=(stop and i == len(chunks) - 1
                                  and j == len(pieces) - 1),
                            skip_group_check=True,
                        )

            def emit_epilogue(blk):
                bi, h = divmod(blk, H)
                s = st[blk]
                rec2 = rcp.tile([128, S], F32, tag="rec2")
                nc.vector.reciprocal_approx_fast(out=rec2, in_=s["r2ps"][:, 0:S])
                ot_sb = outp.tile([128, S], F32, tag="otsb")
                nc.vector.scalar_tensor_tensor(
                    out=ot_sb,
                    in0=s["otps"][:, 0:S],
                    scalar=vtot_sb[:, blk:blk + 1],
                    in1=rec2,
                    op0=ALU.add,
                    op1=ALU.mult,
                )
                nc.vector.memset(ot_sb[:, 0:1], 0.0)
                nc.sync.dma_start(
                    out=out1_d[bi, DK * h:DK * (h + 1), :], in_=ot_sb[0:DK, :]
                )
                nc.sync.dma_start(
                    out=out2_d[bi, DK * h:DK * (h + 1), :],
                    in_=ot_sb[DK:2 * DK, :],
                )

            def emit_A_start(blk):
                bi, h = divmod(blk, H)
                s = st[blk]
                qt_sb = qkp.tile([DK, S], BF16, tag="qt")
                kt_sb = qkp.tile([DK, S], BF16, tag="kt")
                nc.sync.dma_start(out=qt_sb, in_=qt_d[bi, h])
                nc.sync.dma_start(out=kt_sb, in_=kt_d[bi, h])
                vc_sb = vcp.tile([128, NCH, 128], BF16, tag="vc")
                nc.sync.dma_start(
                    out=vc_sb, in_=vcat_d[bi, h].rearrange("c p j -> p c j")
                )
                s["qt"], s["kt"], s["vc"] = qt_sb, kt_sb, vc_sb
                s["e1"] = e1p.tile([128, PACK], BF16, tag="e1", name="e1")
                s["r1ps"] = r_psp.tile([128, S], F32, tag="r1", name="r1ps")

            # ---- software pipeline, 3-deep skew ----
            # iter n runs: A(n) QK/exp1/r1 | C(n-1) mul/exp2/sub |
            # D(n-2) PV/r2 matmuls | epilogue(n-3) recip2/stt/DMA-out.
            # The epilogue leads the iteration so the single-buffered psum
            # tiles (r2/ot) are released long before D(n-2) claims them,
            # and D matmuls are emitted mid-iteration so their psum WAR
            # waits never head-of-line-block the QK stream.
            for n in range(NB + 3):
                a, cn, dn, en = n, n - 1, n - 2, n - 3
                hasA = a < NB
                hasC = 0 <= cn < NB
                hasD = 0 <= dn < NB
                hasE = 0 <= en < NB

                if hasA:
                    emit_A_start(a)
                    emit_qk_group(a, QK_GROUPS[0])
                if hasE:
                    emit_epilogue(en)
                if hasA:
                    emit_qk_group(a, QK_GROUPS[1])
                if hasC:
                    emit_C(cn, 0)
                if hasA:
                    emit_qk_group(a, QK_GROUPS[2])
                if hasD:
                    emit_D_start(dn)
                    emit_pv(dn, [0, 1], stop=False)
                if hasA:
                    emit_qk_group(a, QK_GROUPS[3])
                if hasD:
                    emit_pv(dn, [2, 3, 4], stop=False)
                    emit_r2(dn, [0, 1], stop=False)
                if hasA:
                    emit_r1(a, QK_GROUPS[0] + QK_GROUPS[1],
                            start=True, stop=False)
                    emit_qk_group(a, QK_GROUPS[4])
                if hasC:
                    emit_C(cn, 1)
                if hasD:
                    emit_pv(dn, [5, 6, 7], stop=True)
                    emit_r2(dn, [2, 3, 4], stop=False)
                if hasA:
                    rest = [c for g in QK_GROUPS[2:] for c in g]
                    emit_r1(a, rest, start=False, stop=True)
                    emit_rec1(a)
                if hasD:
                    emit_r2(dn, [5, 6, 7], stop=True)

    nc.compile()
    return nc


_NC_CACHE = None


def _get_nc():
    global _NC_CACHE
    if _NC_CACHE is None:
        _NC_CACHE = build_nc()
    return _NC_CACHE


def make_in_maps(q, k, v1, v2, cm):
    """Full inputs -> per-core input maps (host-side sharding + layout)."""
    q = np.asarray(q, dtype=np.float32).astype(NPBF16)
    k = np.asarray(k, dtype=np.float32).astype(NPBF16)
    v1 = np.asarray(v1, dtype=np.float32)
    v2 = np.asarray(v2, dtype=np.float32)
    cm = np.asarray(cm)

    # additive causal mask for the diagonal block: 0 where k < q else -1e30
    dmask = np.where(
        np.arange(128)[:, None] < np.arange(128)[None, :], 0.0, -1e30
    ).astype(NPBF16)
    ident = np.eye(128, dtype=NPBF16)
    onesd = np.ones((128, 128), NPBF16)
    ind = np.repeat(np.eye(NCH, dtype=np.float32), 128, axis=1).astype(NPBF16)
    cnt = np.full((NCH, 128), float(S), np.float32).astype(NPBF16)

    in_maps = []
    for core in range(NCORES):
        b0 = core * BLOC
        qt = np.ascontiguousarray(
            q[b0:b0 + BLOC].reshape(BLOC, S, H, DK).transpose(0, 2, 3, 1)
        )  # [b, h, dk, s]
        kt = np.ascontiguousarray(
            k[b0:b0 + BLOC].reshape(BLOC, S, H, DK).transpose(0, 2, 3, 1)
        )
        cml = 1.0 - cm[b0:b0 + BLOC].astype(np.float32)  # [b, s] (1-cm)
        v1s = v1[b0:b0 + BLOC].reshape(BLOC, NCH, 128, H, DK).transpose(0, 3, 1, 2, 4)
        v2s = v2[b0:b0 + BLOC].reshape(BLOC, NCH, 128, H, DK).transpose(0, 3, 1, 2, 4)
        vc = np.empty((BLOC, H, NCH, 128, 128), np.float32)
        vc[..., 0:DK] = v1s
        vc[..., DK:2 * DK] = v2s
        # vtot: unmasked total column sums (the "+1" of every key)
        vtot = np.ascontiguousarray(
            vc.astype(np.float64).sum(axis=(2, 3)).astype(np.float32)
        )  # [b,h,128]
        # counter-mask folded into the PV weights
        vcat = np.ascontiguousarray(
            (vc * cml.reshape(BLOC, 1, NCH, 128, 1)).astype(NPBF16)
        )
        cmrep = np.ascontiguousarray(
            np.broadcast_to(
                cml.reshape(BLOC, NCH, 128, 1), (BLOC, NCH, 128, 128)
            ).astype(NPBF16)
        )
        in_maps.append(
            dict(
                qt=qt, kt=kt, vcat=vcat, vtot=vtot, cmrep=cmrep,
                ind=ind, cnt=cnt, dmask=dmask, ident=ident, onesd=onesd,
            )
        )
    return in_maps


def _gather(res):
    out1 = np.concatenate(
        [r["out1t"].transpose(0, 2, 1) for r in res.results], axis=0
    )
    out2 = np.concatenate(
        [r["out2t"].transpose(0, 2, 1) for r in res.results], axis=0
    )
    return np.ascontiguousarray(out1), np.ascontiguousarray(out2)


def kernel(q, k, v1, v2, counter_attention_mask):
    global LAST_RESULTS
    in_maps = make_in_maps(q, k, v1, v2, counter_attention_mask)
    nc = _get_nc()
    res = run_bass_kernel_spmd(
        nc, in_maps, core_ids=list(range(NCORES)), trace=TRACE
    )
    LAST_RESULTS = res
    return _gather(res)
